# revision 28
# baseline (speedup 1.0000x reference)
"""Multi-head causal attention (B=2, S=2048, D=1024, H=16, hd=64) on 8 TRN2
NeuronCores.

Sharding: 2-way batch x 4-way head tensor parallel. Core c handles batch
c//4 and heads 4*(c%4) .. 4*(c%4)+3 (a 256-column feature slice of the QKV
projections / 256 rows of o_W). Each core computes a full [2048, 1024]
partial of its batch's output; the host sums the 4 partials per batch.

All matmuls in fp16 (fp32 PSUM accumulate; validated end-to-end max rel
error ~5e-4 vs the fp32 reference):
  1. Qt/Kt feature-major:  Qt[f, tok] = sum_D wq[D, f] * xT[D, tok]
  2. V row-major (stationary operand = xT chunk), with a 1.0 column
     appended per head ([tok, 65] blocks) so the attention-value matmul
     also produces the softmax denominator.
  3. Causal attention per (head, 512-query chunk), keys in 128 chunks:
       St[k, q] = Kt_chunk.T @ Qt      (scores transposed; the two heads
                                        of a pair run as concurrent
                                        row-tiled 64-contraction matmuls)
       U = exp(St / 8)                 (ACT, scale fused; no max
                                        subtraction -- scores are O(1))
       causal mask: gpsimd affine_select zeroes k > q on the diagonal
       128x128 block; sub-diagonal key chunks are skipped entirely.
     AV row-major per 128-query subchunk, U slice as the stationary:
       AO_aug[q, 65] += U_i[:, qslice].T @ V_aug[i]   (col 64 = denom)
     Normalize: rec = 1/AO_aug[:, 64] (DVE [128,1]), then
     tensor_scalar multiply (per-partition broadcast).
  4. AO transposed back to feature-major via PE transpose (fp16), then
     y[tok, :] = sum_f AOt[f-chunk, tok].T @ wo[f-chunk, :]

Schedule (the key to the speedup vs a phase-by-phase layout): the
attention exp stream costs ~75us of ACTIVATE, comparable to the ~110us
of total PE matmul work, so everything is software-pipelined into one
continuous stream: per attention unit (query-chunk j, head pair), the
QK+exp emission is interleaved with (a) the previous unit's AV matmuls
(pumped at ~10 steps per exp so they finish within the unit) and
(b) filler work from a generator FIFO -- the next token chunk's QKV
projections and the previous chunk's output projection.  Forced drains
at unit boundaries establish emission-order prerequisites (the Tile
dependency graph is defined by emission order).  The last unit gets a
custom endgame: only exp 15 is held back; the AV groups, second-pair
transposes, and output projection slices chase it so that after the
last exp only the final query subchunk's true dependency chain
remains.

Startup: the input DMA fabric is bandwidth-serialized (~18us for the
full 6MB), so transfers are strictly need-ordered and the critical
prefix gating the first projection matmul (wq/wk f=0 halves + the
first half of token chunk 0) is trimmed to 8KB/partition.  A burst of
dummy matmuls keeps the PE busy (and the HAM clock gate open) through
the ~7us framework preamble + prefix DMA, so the real projections run
at 2.4 GHz from their first instruction.  Non-endgame AO transposes
ride the DMA XBAR (sync ring, idle mid-kernel) instead of the PE; the
y output is f16 (host sums partials in f32), halving the output DMA
bytes.  A ~7.5us framework epilogue (semaphore teardown) after the
last DMA is flat in everything this kernel controls.
"""

import numpy as np

import concourse.mybir as mybir
import concourse.tile as tile
from concourse import bacc
from concourse.bass_utils import run_bass_kernel_spmd

F32 = mybir.dt.float32
F16 = mybir.dt.float16

S = 2048          # tokens per batch (= per core)
D = 1024          # model dim
HD = 64           # head dim
CORE_HEADS = 4    # heads per core
CF = CORE_HEADS * HD  # feature columns per core (256)
QC = 512          # query chunk (QK/exp granularity)
KC = 128          # key chunk
NQ = S // QC      # 4 query chunks
NK = S // KC      # 16 key chunks
ND = D // 128     # 8 contraction chunks

_CACHE = {}


def build_nc():
    nc = bacc.Bacc()
    xT = nc.dram_tensor("xT", [D, S], F16, kind="ExternalInput")
    wq = nc.dram_tensor("wq", [D, CF], F16, kind="ExternalInput")
    wk = nc.dram_tensor("wk", [D, CF], F16, kind="ExternalInput")
    wv = nc.dram_tensor("wv", [D, CF], F16, kind="ExternalInput")
    wo = nc.dram_tensor("wo", [CF, D], F16, kind="ExternalInput")
    y = nc.dram_tensor("y", [S, D], F16, kind="ExternalOutput")

    with tile.TileContext(nc) as tc:
        with (
            tc.tile_pool(name="big", bufs=1) as big,
            tc.tile_pool(name="w", bufs=1) as wpool,
            tc.tile_pool(name="u", bufs=34) as upool,
            tc.tile_pool(name="aoq", bufs=16) as aoqpool,
            tc.tile_pool(name="aot", bufs=3) as aotpool,
            tc.tile_pool(name="sm", bufs=8) as smpool,
            tc.tile_pool(name="ost", bufs=4) as ostpool,
            tc.tile_pool(name="ps", bufs=2, space="PSUM") as psp,
            tc.tile_pool(name="pav", bufs=2, space="PSUM") as pavp,
            tc.tile_pool(name="pt", bufs=2, space="PSUM") as ptp,
        ):
            # ---- constants ----
            ident = wpool.tile([128, 128], F16, tag="ident")
            nc.gpsimd.memset(ident[:], 0.0)
            nc.gpsimd.affine_select(
                out=ident[:], in_=ident[:],
                compare_op=mybir.AluOpType.not_equal, fill=1.0,
                base=0, channel_multiplier=1, pattern=[[-1, 128]],
            )

            # ---- weight + activation loads, startup-ordered ----
            # First projection matmul needs xs[:, :, 0:512] + wq/wk; those
            # gens go first, spread across all four HWDGE/SWDGE rings so the
            # descriptor generations overlap (the two-ring version serialized
            # ~10us of gens on sync+scalar).  scalar's ring frees up before
            # the first exp; wo (needed last) goes via gpsimd SWDGE.
            wq_sb = wpool.tile([128, ND, CF], F16, tag="wq")
            wk_sb = wpool.tile([128, ND, CF], F16, tag="wk")
            wv_sb = wpool.tile([128, ND, CF], F16, tag="wv")
            wo_sb = wpool.tile([128, 2, D], F16, tag="wo")
            xs = big.tile([128, ND, S], F16, tag="xs")
            xTv = xT.rearrange("(n p) m -> p n m", p=128)
            # The DMA fabric is bandwidth-serialized (~0.39 ns per byte per
            # partition aggregate; the full 6MB input takes ~18us), so ring
            # choice doesn't matter for throughput — only FIFO ORDER does.
            # Transfers are strictly need-ordered, and the critical prefix
            # that gates the first real matmul (wq/wk f=0 halves + the
            # first half of token-chunk 0) is trimmed to 8KB/partition
            # (~3.1us), putting real PE work at ~+12us.  Every extra DMA
            # instruction also costs ~0.15us in the framework epilogue
            # (per-DMA semaphore teardown), so transfers are kept coarse.
            wqv = wq.rearrange("(n p) m -> p n m", p=128)
            wkv = wk.rearrange("(n p) m -> p n m", p=128)
            nc.sync.dma_start(wq_sb[:, :, 0:128], wqv[:, :, 0:128])
            nc.scalar.dma_start(wk_sb[:, :, 0:128], wkv[:, :, 0:128])
            nc.sync.dma_start(xs[:, 0:4, 0:QC], xTv[:, 0:4, 0:QC])
            nc.scalar.dma_start(xs[:, 4:8, 0:QC], xTv[:, 4:8, 0:QC])
            nc.sync.dma_start(wq_sb[:, :, 128:CF], wqv[:, :, 128:CF])
            nc.scalar.dma_start(wk_sb[:, :, 128:CF], wkv[:, :, 128:CF])
            nc.scalar.dma_start(wv_sb[:], wv.rearrange("(n p) m -> p n m", p=128))
            nc.sync.dma_start(xs[:, :, QC:2 * QC], xTv[:, :, QC:2 * QC])
            nc.scalar.dma_start(xs[:, :, 2 * QC:3 * QC], xTv[:, :, 2 * QC:3 * QC])
            nc.sync.dma_start(wo_sb[:], wo.rearrange("(b p) n -> p b n", p=128))
            nc.scalar.dma_start(xs[:, :, 3 * QC:4 * QC], xTv[:, :, 3 * QC:4 * QC])

            # qt/kt: [128, 2, S]: partition = feat % 128 (2 heads), block =
            # feat // 128 (head pair), col = token.
            qt = big.tile([128, 2, S], F16, tag="qt")
            kt = big.tile([128, 2, S], F16, tag="kt")
            v_sb = big.tile([128, NK, CORE_HEADS * (HD + 1)], F16, tag="v")
            nc.vector.memset(
                v_sb[:].rearrange("p n (h c) -> p n h c", c=HD + 1)[:, :, :, HD:],
                1.0,
            )

            # ---- PE warm-up: dummy matmuls keep the PE busy (and the HAM
            # clock gate open) through the framework preamble + input DMA
            # (~13us), so the real projections run at 2.4 GHz from their
            # start.
            ps_warm = ptp.tile([128, 64], F32, tag="t", name="ps_warm")
            for _ in range(110):
                nc.tensor.matmul(ps_warm[0:64, :], ident[:, 0:64],
                                 ident[:, 0:64], start=True, stop=True,
                                 skip_group_check=True)

            # ---- projection emitters (generators: yield after each PE op)
            def projqk_gen(t, f):
                ps_q = ptp.tile([128, QC], F32, tag="t", name=f"pq{t}_{f}")
                ps_k = ptp.tile([128, QC], F32, tag="t", name=f"pk{t}_{f}")
                for k in range(ND):
                    with nc.named_scope("mm_projqk"):
                        nc.tensor.matmul(
                            ps_q[:],
                            wq_sb[:, k, 128 * f:128 * (f + 1)],
                            xs[:, k, QC * t:QC * (t + 1)],
                            start=(k == 0), stop=(k == ND - 1),
                        )
                    yield
                    with nc.named_scope("mm_projqk"):
                        nc.tensor.matmul(
                            ps_k[:],
                            wk_sb[:, k, 128 * f:128 * (f + 1)],
                            xs[:, k, QC * t:QC * (t + 1)],
                            start=(k == 0), stop=(k == ND - 1),
                        )
                    yield
                nc.vector.tensor_copy(qt[:, f, QC * t:QC * (t + 1)], ps_q[:])
                nc.vector.tensor_copy(kt[:, f, QC * t:QC * (t + 1)], ps_k[:])

            def projv_gen(t):
                for tt in range(4 * t, 4 * t + 4):
                    ps = ptp.tile([128, CF], F32, tag="t", name=f"pv{tt}")
                    for k in range(ND):
                        with nc.named_scope("mm_projv"):
                            nc.tensor.matmul(
                                ps[:],
                                xs[:, k, KC * tt:KC * (tt + 1)],
                                wv_sb[:, k, :],
                                start=(k == 0), stop=(k == ND - 1),
                            )
                        yield
                    nc.vector.tensor_copy(
                        v_sb[:, tt, :].rearrange("p (h c) -> p h c", c=HD + 1)[:, :, :HD],
                        ps[:].rearrange("p (h c) -> p h c", c=HD),
                    )

            # ---- attention units ----
            units = [(j, pair) for j in range(NQ) for pair in range(2)]
            us = {}
            ao_q = {}

            def emit_A(unit, i):
                # Both heads' scores land in one 2-bank PSUM tile so a
                # single ACTIVATE (and a single affine_select) covers the
                # pair — halves the fixed 352-cycle ACT pipeline overhead.
                # The two 64-contraction matmuls run concurrently via PE
                # row tiling (auto tile_position from base partitions 0/64).
                j, pair = unit
                t = i - 4 * j
                qo = max(0, KC * t)
                w = QC - qo
                ps_s = psp.tile([128, 2, QC], F32, tag="s", name=f"s{j}_{pair}_{i}")
                for hx, h in enumerate((2 * pair, 2 * pair + 1)):
                    hp = 64 * (h % 2)
                    with nc.named_scope("mm_qk"):
                        nc.tensor.matmul(
                            ps_s[:, hx, 0:w],
                            kt[hp:hp + 64, pair, KC * i:KC * (i + 1)],
                            qt[hp:hp + 64, pair, QC * j + qo:QC * (j + 1)],
                            start=True, stop=True,
                            skip_group_check=True,
                        )
                u = upool.tile([128, 2, w], F16, tag="u", name=f"u{j}_{pair}_{i}")
                nc.scalar.activation(
                    u[:], ps_s[:, :, 0:w],
                    mybir.ActivationFunctionType.Exp, scale=0.125,
                )
                if t >= 0:
                    nc.gpsimd.affine_select(
                        out=u[:, :, 0:KC], in_=u[:, :, 0:KC],
                        compare_op=mybir.AluOpType.is_ge, fill=0.0,
                        base=0, channel_multiplier=-1,
                        pattern=[[0, 2], [1, KC]],
                    )
                us[unit, i] = (u, qo)

            def av_gen(unit):
                """Generator: AV matmuls for one unit, yielding after each
                key-chunk step; norms emitted at each query-subchunk's end.
                One PSUM bank per accumulation group (bank-granular
                start/stop semantics)."""
                j, pair = unit
                nk = 4 * j + 4
                if j not in ao_q:
                    ao_q[j] = [aoqpool.tile([128, CF], F16, tag="aoq",
                                            name=f"ao_q{j}_{qq}")
                               for qq in range(4)]
                for qq in range(4):
                    for hx, h in enumerate((2 * pair, 2 * pair + 1)):
                        av = pavp.tile([128, HD + 1], F32, tag="av",
                                       name=f"av{j}_{h}_{qq}")
                        last = 4 * j + qq
                        for i in range(last + 1):
                            u, qo = us[unit, i]
                            with nc.named_scope("mm_av"):
                                nc.tensor.matmul(
                                    av[:],
                                    u[:, hx, KC * qq - qo:KC * (qq + 1) - qo],
                                    v_sb[:, i, 65 * h:65 * h + 65],
                                    start=(i == 0), stop=(i == last),
                                )
                            yield
                        rec = smpool.tile([128, 1], F32, tag="rec",
                                          name=f"rec{j}_{h}_{qq}")
                        nc.vector.reciprocal(rec[:], av[:, HD:HD + 1])
                        nc.vector.tensor_scalar_mul(
                            ao_q[j][qq][:, HD * h:HD * (h + 1)],
                            av[:, 0:HD], rec[:],
                        )
                        yield ("qqdone", qq) if hx == 1 else None
                for i in range(nk):
                    us.pop((unit, i), None)

            def out_tpose(j, aot, qq, b, on_scalar=False):
                ps_t = ptp.tile([128, 128], F16, tag="t",
                                name=f"pt{j}_{qq}_{b}")
                with nc.named_scope("mm_tpose"):
                    nc.tensor.transpose(
                        ps_t[:], ao_q[j][qq][:, 128 * b:128 * (b + 1)],
                        ident[:],
                    )
                if on_scalar:
                    nc.scalar.copy(aot[:, b, KC * qq:KC * (qq + 1)], ps_t[:])
                else:
                    nc.vector.tensor_copy(
                        aot[:, b, KC * qq:KC * (qq + 1)], ps_t[:],
                    )

            def out_tpose_dma(j, aot, qq, b):
                # DMA XBAR transpose: frees the PE (24 transposes ~2.7us) and
                # the DVE copies (~10us); the sync ring is idle mid-kernel and
                # the framework epilogue cost is flat in DMA count.  Only off
                # the endgame path (latency ~1.5us vs ~0.3us for PE).
                nc.sync.dma_start(
                    aot[:, b, KC * qq:KC * (qq + 1)],
                    ao_q[j][qq][:, 128 * b:128 * (b + 1)],
                    transpose=True,
                )

            yv = y.rearrange("(c p) d -> p c d", p=128)

            def out_oproj(j, aot, tt4, on_scalar=False):
                # ost (and y) are f16: halves the output DMA bytes; the host
                # sums the four per-batch partials in f32.
                ost = ostpool.tile([128, D], F16, tag="ost",
                                   name=f"ost{j}_{tt4}")
                for n in range(2):
                    ps_o = ptp.tile([128, QC], F32, tag="t",
                                    name=f"po{j}_{tt4}_{n}")
                    for b in range(2):
                        with nc.named_scope("mm_oproj"):
                            nc.tensor.matmul(
                                ps_o[:],
                                aot[:, b, KC * tt4:KC * (tt4 + 1)],
                                wo_sb[:, b, 512 * n:512 * (n + 1)],
                                start=(b == 0), stop=(b == 1),
                            )
                        yield
                    if on_scalar:
                        # ACT is idle after its last exp; draining the final
                        # output tiles there keeps the endgame's DVE chain
                        # (norms) and the drain in parallel.  The two halves
                        # ship on different rings so their gens overlap.
                        nc.scalar.copy(ost[:, 512 * n:512 * (n + 1)], ps_o[:])
                        ring = nc.sync if n == 0 else nc.scalar
                        ring.dma_start(
                            yv[:, 4 * j + tt4, 512 * n:512 * (n + 1)],
                            ost[:, 512 * n:512 * (n + 1)],
                        )
                    else:
                        nc.vector.tensor_copy(ost[:, 512 * n:512 * (n + 1)], ps_o[:])
                if not on_scalar:
                    nc.sync.dma_start(yv[:, 4 * j + tt4, :], ost[:])

            def out_gen(j):
                aot = aotpool.tile([128, 2, QC], F16, tag="aot", name=f"aot{j}")
                for qq in range(4):
                    for b in range(2):
                        out_tpose_dma(j, aot, qq, b)
                    yield
                for tt4 in range(QC // KC):
                    yield from out_oproj(j, aot, tt4)

            # ---- interleaved schedule ----
            # projqk(0, f=0) runs alone (nothing to overlap yet); each
            # attention unit's QK+exp stream is then interleaved with
            # (a) the previous unit's AV matmuls (pumped fast enough to
            # finish within the unit, avoiding a boundary drain) and
            # (b) "big" filler work from a FIFO of generators: the
            # remaining projections and the previous chunks' output
            # projections, keeping the PE busy while ACT runs exp.
            # Forced drains at unit boundaries guarantee emission-order
            # prerequisites (the dependency graph is defined by emission
            # order).  The last unit gets a custom endgame: its own AV
            # groups and out(3) chase the final exps instead of running
            # serially after them.
            work = []          # FIFO of big-step generators

            def pump_work(n):
                while n > 0 and work:
                    try:
                        next(work[0])
                        n -= 1
                    except StopIteration:
                        work.pop(0)

            def drain(g):
                if g in work:
                    work.remove(g)
                for _ in g:
                    pass

            for _ in projqk_gen(0, 0):
                pass
            pqk = {}
            pv = {}
            pqk[(0, 1)] = projqk_gen(0, 1)
            pv[0] = projv_gen(0)
            work.append(pqk[(0, 1)])
            work.append(pv[0])

            units = [(j, pair) for j in range(NQ) for pair in range(2)]
            prev_av = [None]

            def pump_av(n):
                k = 0
                while k < n and prev_av[0] is not None:
                    try:
                        next(prev_av[0])
                        k += 1
                    except StopIteration:
                        prev_av[0] = None
                return k

            for unit in units:
                j, pair = unit
                last_unit = unit == (NQ - 1, 1)
                if pair == 0 and j + 1 < NQ:
                    pqk[(j + 1, 0)] = projqk_gen(j + 1, 0)
                    pqk[(j + 1, 1)] = projqk_gen(j + 1, 1)
                    pv[j + 1] = projv_gen(j + 1)
                    work.append(pqk[(j + 1, 0)])
                    work.append(pqk[(j + 1, 1)])
                    work.append(pv[j + 1])
                if pair == 1 and j >= 1:
                    # ao_q[j-1] is fully written only once av_gen((j-1, 1))
                    # has drained (end of unit (j, 0)); emitting out_gen
                    # earlier would read-before-write.
                    work.append(out_gen(j - 1))
                ni = 4 * j + 4
                for i in range(ni - 1 if last_unit else ni):
                    # Filler (long 213ns projection streams) goes right
                    # BEFORE the qk pair so its 64-row LDWEIGHTS hides
                    # under a long stream instead of trailing the dense
                    # 27ns AV burst (whose LDW port is saturated).
                    pump_work(3)
                    emit_A(unit, i)
                    adv = pump_av(10)
                    pump_work(max(0, (10 - adv) // 3 - 1))
                while prev_av[0] is not None:
                    pump_av(64)
                if pair == 0:
                    # unit (j, 1) needs qt/kt f=1; av_gen((j, 0)) (pumped
                    # during (j, 1)) reads v chunks of chunk j.
                    drain(pqk[(j, 1)])
                    drain(pv[j])
                else:
                    if (j + 1, 0) in pqk:
                        drain(pqk[(j + 1, 0)])
                prev_av[0] = av_gen(unit)
            # Endgame for unit (3, 1): exps 0..14 ran in the normal loop;
            # only exp 15 (the last diagonal block) is outstanding, emitted
            # once the av stream has pumped past group qq=0 h0 (by which
            # point exp 13's PSUM score buffer is free, so qk15 never
            # stalls the PE queue).  As each query subchunk completes, its
            # second-pair transpose and output-projection slice chase it —
            # after the last exp only the true dependency chain of the
            # final subchunk remains.
            j3 = NQ - 1
            aot3 = aotpool.tile([128, 2, QC], F16, tag="aot", name="aot3")
            for qq in range(4):
                # PE transposes here: a DMA version queues behind
                # out_gen(2)'s ring traffic and arrives too late,
                # serializing the endgame oprojs (measured +8us).
                out_tpose(j3, aot3, qq, 0)
            av31 = prev_av[0]
            steps = 0
            for ev in av31:
                steps += 1
                if steps == 13:
                    emit_A((j3, 1), 4 * j3 + 3)
                if isinstance(ev, tuple) and ev[0] == "qqdone":
                    qq = ev[1]
                    out_tpose(j3, aot3, qq, 1, on_scalar=True)
                    for _ in out_oproj(j3, aot3, qq, on_scalar=True):
                        pass
                pump_work(2)
            for g in list(work):
                drain(g)
    nc.compile()
    return nc


def _get_nc():
    if "nc" not in _CACHE:
        _CACHE["nc"] = build_nc()
    return _CACHE["nc"]


def make_in_maps(x, q_W, k_W, v_W, o_W):
    x = np.asarray(x, dtype=np.float32)
    in_maps = []
    xTs = [np.ascontiguousarray(x[b].T).astype(np.float16) for b in range(2)]
    for c in range(8):
        b, g = c // 4, c % 4
        fs = slice(CF * g, CF * (g + 1))
        in_maps.append({
            "xT": xTs[b],
            "wq": np.ascontiguousarray(np.asarray(q_W, np.float32)[fs].T).astype(np.float16),
            "wk": np.ascontiguousarray(np.asarray(k_W, np.float32)[fs].T).astype(np.float16),
            "wv": np.ascontiguousarray(np.asarray(v_W, np.float32)[fs].T).astype(np.float16),
            "wo": np.ascontiguousarray(np.asarray(o_W, np.float32)[:, fs].T).astype(np.float16),
        })
    return in_maps


def kernel(x, q_W, k_W, v_W, o_W, trace=False):
    nc = _get_nc()
    in_maps = make_in_maps(x, q_W, k_W, v_W, o_W)
    res = run_bass_kernel_spmd(nc, in_maps, core_ids=list(range(8)),
                               trace=trace)
    _CACHE["last_results"] = res
    ys = [np.asarray(res.results[c]["y"], np.float32) for c in range(8)]
    out = np.stack([
        ys[0] + ys[1] + ys[2] + ys[3],
        ys[4] + ys[5] + ys[6] + ys[7],
    ]).astype(np.float32)
    return out



# revision 29
# speedup vs baseline: 1.0407x; 1.0407x over previous
"""Multi-head causal attention (B=2, S=2048, D=1024, H=16, hd=64) on 8 TRN2
NeuronCores.

Sharding: 2-way batch x 4-way head tensor parallel. Core c handles batch
c//4 and heads 4*(c%4) .. 4*(c%4)+3 (a 256-column feature slice of the QKV
projections / 256 rows of o_W). Each core computes a full [2048, 1024]
partial of its batch's output; the host sums the 4 partials per batch.

All matmuls in fp16 (fp32 PSUM accumulate; validated end-to-end max rel
error ~5e-4 vs the fp32 reference):
  1. Qt/Kt feature-major:  Qt[f, tok] = sum_D wq[D, f] * xT[D, tok]
  2. V row-major (stationary operand = xT chunk), with a 1.0 column
     appended per head ([tok, 65] blocks) so the attention-value matmul
     also produces the softmax denominator.
  3. Causal attention per (head, 512-query chunk), keys in 128 chunks:
       St[k, q] = Kt_chunk.T @ Qt      (scores transposed; the two heads
                                        of a pair run as concurrent
                                        row-tiled 64-contraction matmuls)
       U = exp(St / 8)                 (ACT, scale fused; no max
                                        subtraction -- scores are O(1))
       causal mask: gpsimd affine_select zeroes k > q on the diagonal
       128x128 block; sub-diagonal key chunks are skipped entirely.
     AV row-major per 128-query subchunk, U slice as the stationary:
       AO_aug[q, 65] += U_i[:, qslice].T @ V_aug[i]   (col 64 = denom)
     Normalize: rec = 1/AO_aug[:, 64] (DVE [128,1]), then
     tensor_scalar multiply (per-partition broadcast).
  4. AO transposed back to feature-major via PE transpose (fp16), then
     y[tok, :] = sum_f AOt[f-chunk, tok].T @ wo[f-chunk, :]

Schedule (the key to the speedup vs a phase-by-phase layout): the
attention exp stream costs ~75us of ACTIVATE, comparable to the ~110us
of total PE matmul work, so everything is software-pipelined into one
continuous stream: per attention unit (query-chunk j, head pair), the
QK+exp emission is interleaved with (a) the previous unit's AV matmuls
(pumped at ~10 steps per exp so they finish within the unit) and
(b) filler work from a generator FIFO -- the next token chunk's QKV
projections and the previous chunk's output projection.  Forced drains
at unit boundaries establish emission-order prerequisites (the Tile
dependency graph is defined by emission order).  The last unit gets a
custom endgame: only exp 15 is held back; the AV groups, second-pair
transposes, and output projection slices chase it so that after the
last exp only the final query subchunk's true dependency chain
remains.

Startup: the input DMA fabric is bandwidth-serialized (~18us for the
full 6MB), so transfers are strictly need-ordered and the critical
prefix gating the first projection matmul (wq/wk f=0 halves + the
first half of token chunk 0) is trimmed to 8KB/partition.  A burst of
dummy matmuls keeps the PE busy (and the HAM clock gate open) through
the ~7us framework preamble + prefix DMA, so the real projections run
at 2.4 GHz from their first instruction.  Non-endgame AO transposes
ride the DMA XBAR (sync ring, idle mid-kernel) instead of the PE; the
y output is f16 (host sums partials in f32), halving the output DMA
bytes.  A ~7.5us framework epilogue (semaphore teardown) after the
last DMA is flat in everything this kernel controls.
"""

import numpy as np

import concourse.mybir as mybir
import concourse.tile as tile
from concourse import bacc
from concourse.bass_utils import run_bass_kernel_spmd

F32 = mybir.dt.float32
F16 = mybir.dt.float16

S = 2048          # tokens per batch (= per core)
D = 1024          # model dim
HD = 64           # head dim
CORE_HEADS = 4    # heads per core
CF = CORE_HEADS * HD  # feature columns per core (256)
QC = 512          # query chunk (QK/exp granularity)
KC = 128          # key chunk
NQ = S // QC      # 4 query chunks
NK = S // KC      # 16 key chunks
ND = D // 128     # 8 contraction chunks

_CACHE = {}


def build_nc():
    nc = bacc.Bacc()
    xT = nc.dram_tensor("xT", [D, S], F16, kind="ExternalInput")
    wq = nc.dram_tensor("wq", [D, CF], F16, kind="ExternalInput")
    wk = nc.dram_tensor("wk", [D, CF], F16, kind="ExternalInput")
    wv = nc.dram_tensor("wv", [D, CF], F16, kind="ExternalInput")
    wo = nc.dram_tensor("wo", [CF, D], F16, kind="ExternalInput")
    y = nc.dram_tensor("y", [S, D], F16, kind="ExternalOutput")

    with tile.TileContext(nc) as tc:
        with (
            tc.tile_pool(name="big", bufs=1) as big,
            tc.tile_pool(name="w", bufs=1) as wpool,
            tc.tile_pool(name="u", bufs=34) as upool,
            tc.tile_pool(name="aoq", bufs=16) as aoqpool,
            tc.tile_pool(name="aot", bufs=3) as aotpool,
            tc.tile_pool(name="sm", bufs=8) as smpool,
            tc.tile_pool(name="ost", bufs=4) as ostpool,
            tc.tile_pool(name="ps", bufs=2, space="PSUM") as psp,
            tc.tile_pool(name="pav", bufs=2, space="PSUM") as pavp,
            tc.tile_pool(name="pt", bufs=2, space="PSUM") as ptp,
        ):
            # ---- constants ----
            ident = wpool.tile([128, 128], F16, tag="ident")
            nc.gpsimd.memset(ident[:], 0.0)
            nc.gpsimd.affine_select(
                out=ident[:], in_=ident[:],
                compare_op=mybir.AluOpType.not_equal, fill=1.0,
                base=0, channel_multiplier=1, pattern=[[-1, 128]],
            )

            # ---- weight + activation loads, startup-ordered ----
            # First projection matmul needs xs[:, :, 0:512] + wq/wk; those
            # gens go first, spread across all four HWDGE/SWDGE rings so the
            # descriptor generations overlap (the two-ring version serialized
            # ~10us of gens on sync+scalar).  scalar's ring frees up before
            # the first exp; wo (needed last) goes via gpsimd SWDGE.
            wq_sb = wpool.tile([128, ND, CF], F16, tag="wq")
            wk_sb = wpool.tile([128, ND, CF], F16, tag="wk")
            wv_sb = wpool.tile([128, ND, CF], F16, tag="wv")
            wo_sb = wpool.tile([128, 2, D], F16, tag="wo")
            xs = big.tile([128, ND, S], F16, tag="xs")
            xTv = xT.rearrange("(n p) m -> p n m", p=128)
            # The DMA fabric is bandwidth-serialized (~0.39 ns per byte per
            # partition aggregate; the full 6MB input takes ~18us), so ring
            # choice doesn't matter for throughput — only FIFO ORDER does.
            # Transfers are strictly need-ordered, and the critical prefix
            # that gates the first real matmul (wq/wk f=0 halves + the
            # first half of token-chunk 0) is trimmed to 8KB/partition
            # (~3.1us), putting real PE work at ~+12us.  Every extra DMA
            # instruction also costs ~0.15us in the framework epilogue
            # (per-DMA semaphore teardown), so transfers are kept coarse.
            wqv = wq.rearrange("(n p) m -> p n m", p=128)
            wkv = wk.rearrange("(n p) m -> p n m", p=128)
            nc.sync.dma_start(wq_sb[:, :, 0:128], wqv[:, :, 0:128])
            nc.scalar.dma_start(wk_sb[:, :, 0:128], wkv[:, :, 0:128])
            nc.sync.dma_start(xs[:, 0:4, 0:QC], xTv[:, 0:4, 0:QC])
            nc.scalar.dma_start(xs[:, 4:8, 0:QC], xTv[:, 4:8, 0:QC])
            nc.sync.dma_start(wq_sb[:, :, 128:CF], wqv[:, :, 128:CF])
            nc.scalar.dma_start(wk_sb[:, :, 128:CF], wkv[:, :, 128:CF])
            nc.scalar.dma_start(wv_sb[:], wv.rearrange("(n p) m -> p n m", p=128))
            nc.sync.dma_start(xs[:, :, QC:2 * QC], xTv[:, :, QC:2 * QC])
            nc.scalar.dma_start(xs[:, :, 2 * QC:3 * QC], xTv[:, :, 2 * QC:3 * QC])
            nc.sync.dma_start(wo_sb[:], wo.rearrange("(b p) n -> p b n", p=128))
            nc.scalar.dma_start(xs[:, :, 3 * QC:4 * QC], xTv[:, :, 3 * QC:4 * QC])

            # qt/kt: [128, 2, S]: partition = feat % 128 (2 heads), block =
            # feat // 128 (head pair), col = token.
            qt = big.tile([128, 2, S], F16, tag="qt")
            kt = big.tile([128, 2, S], F16, tag="kt")
            v_sb = big.tile([128, NK, CORE_HEADS * (HD + 1)], F16, tag="v")
            nc.vector.memset(
                v_sb[:].rearrange("p n (h c) -> p n h c", c=HD + 1)[:, :, :, HD:],
                1.0,
            )

            # ---- PE warm-up: dummy matmuls keep the PE busy (and the HAM
            # clock gate open) through the framework preamble + input DMA
            # (~13us), so the real projections run at 2.4 GHz from their
            # start.
            ps_warm = ptp.tile([128, 64], F32, tag="t", name="ps_warm")
            for _ in range(110):
                nc.tensor.matmul(ps_warm[0:64, :], ident[:, 0:64],
                                 ident[:, 0:64], start=True, stop=True,
                                 skip_group_check=True)

            # ---- projection emitters (generators: yield after each PE op)
            def projqk_gen(t, f):
                ps_q = ptp.tile([128, QC], F32, tag="t", name=f"pq{t}_{f}")
                ps_k = ptp.tile([128, QC], F32, tag="t", name=f"pk{t}_{f}")
                for k in range(ND):
                    with nc.named_scope("mm_projqk"):
                        nc.tensor.matmul(
                            ps_q[:],
                            wq_sb[:, k, 128 * f:128 * (f + 1)],
                            xs[:, k, QC * t:QC * (t + 1)],
                            start=(k == 0), stop=(k == ND - 1),
                        )
                    yield
                    with nc.named_scope("mm_projqk"):
                        nc.tensor.matmul(
                            ps_k[:],
                            wk_sb[:, k, 128 * f:128 * (f + 1)],
                            xs[:, k, QC * t:QC * (t + 1)],
                            start=(k == 0), stop=(k == ND - 1),
                        )
                    yield
                nc.vector.tensor_copy(qt[:, f, QC * t:QC * (t + 1)], ps_q[:])
                nc.vector.tensor_copy(kt[:, f, QC * t:QC * (t + 1)], ps_k[:])

            def projv_gen(t):
                for tt in range(4 * t, 4 * t + 4):
                    ps = ptp.tile([128, CF], F32, tag="t", name=f"pv{tt}")
                    for k in range(ND):
                        with nc.named_scope("mm_projv"):
                            nc.tensor.matmul(
                                ps[:],
                                xs[:, k, KC * tt:KC * (tt + 1)],
                                wv_sb[:, k, :],
                                start=(k == 0), stop=(k == ND - 1),
                            )
                        yield
                    nc.vector.tensor_copy(
                        v_sb[:, tt, :].rearrange("p (h c) -> p h c", c=HD + 1)[:, :, :HD],
                        ps[:].rearrange("p (h c) -> p h c", c=HD),
                    )

            # ---- attention units ----
            units = [(j, pair) for j in range(NQ) for pair in range(2)]
            us = {}
            ao_q = {}

            def emit_A(unit, i):
                # Both heads' scores land in one 2-bank PSUM tile so a
                # single ACTIVATE (and a single affine_select) covers the
                # pair — halves the fixed 352-cycle ACT pipeline overhead.
                # The two 64-contraction matmuls run concurrently via PE
                # row tiling (auto tile_position from base partitions 0/64).
                j, pair = unit
                t = i - 4 * j
                qo = max(0, KC * t)
                w = QC - qo
                ps_s = psp.tile([128, 2, QC], F32, tag="s", name=f"s{j}_{pair}_{i}")
                for hx, h in enumerate((2 * pair, 2 * pair + 1)):
                    hp = 64 * (h % 2)
                    with nc.named_scope("mm_qk"):
                        nc.tensor.matmul(
                            ps_s[:, hx, 0:w],
                            kt[hp:hp + 64, pair, KC * i:KC * (i + 1)],
                            qt[hp:hp + 64, pair, QC * j + qo:QC * (j + 1)],
                            start=True, stop=True,
                            skip_group_check=True,
                        )
                u = upool.tile([128, 2, w], F16, tag="u", name=f"u{j}_{pair}_{i}")
                nc.scalar.activation(
                    u[:], ps_s[:, :, 0:w],
                    mybir.ActivationFunctionType.Exp, scale=0.125,
                )
                if t >= 0:
                    nc.gpsimd.affine_select(
                        out=u[:, :, 0:KC], in_=u[:, :, 0:KC],
                        compare_op=mybir.AluOpType.is_ge, fill=0.0,
                        base=0, channel_multiplier=-1,
                        pattern=[[0, 2], [1, KC]],
                    )
                us[unit, i] = (u, qo)

            def av_gen(unit):
                """Generator: AV matmuls for one unit, yielding after each
                key-chunk step; norms emitted at each query-subchunk's end.
                One PSUM bank per accumulation group (bank-granular
                start/stop semantics)."""
                j, pair = unit
                nk = 4 * j + 4
                if j not in ao_q:
                    ao_q[j] = [aoqpool.tile([128, CF], F16, tag="aoq",
                                            name=f"ao_q{j}_{qq}")
                               for qq in range(4)]
                for qq in range(4):
                    for hx, h in enumerate((2 * pair, 2 * pair + 1)):
                        av = pavp.tile([128, HD + 1], F32, tag="av",
                                       name=f"av{j}_{h}_{qq}")
                        last = 4 * j + qq
                        for i in range(last + 1):
                            u, qo = us[unit, i]
                            with nc.named_scope("mm_av"):
                                nc.tensor.matmul(
                                    av[:],
                                    u[:, hx, KC * qq - qo:KC * (qq + 1) - qo],
                                    v_sb[:, i, 65 * h:65 * h + 65],
                                    start=(i == 0), stop=(i == last),
                                )
                            yield
                        rec = smpool.tile([128, 1], F32, tag="rec",
                                          name=f"rec{j}_{h}_{qq}")
                        nc.vector.reciprocal(rec[:], av[:, HD:HD + 1])
                        nc.vector.tensor_scalar_mul(
                            ao_q[j][qq][:, HD * h:HD * (h + 1)],
                            av[:, 0:HD], rec[:],
                        )
                        yield ("qqdone", qq) if hx == 1 else None
                for i in range(nk):
                    us.pop((unit, i), None)

            def out_tpose(j, aot, qq, b, on_scalar=False):
                ps_t = ptp.tile([128, 128], F16, tag="t",
                                name=f"pt{j}_{qq}_{b}")
                with nc.named_scope("mm_tpose"):
                    nc.tensor.transpose(
                        ps_t[:], ao_q[j][qq][:, 128 * b:128 * (b + 1)],
                        ident[:],
                    )
                if on_scalar:
                    nc.scalar.copy(aot[:, b, KC * qq:KC * (qq + 1)], ps_t[:])
                else:
                    nc.vector.tensor_copy(
                        aot[:, b, KC * qq:KC * (qq + 1)], ps_t[:],
                    )

            def out_tpose_dma(j, aot, qq, b):
                # DMA XBAR transpose: frees the PE (24 transposes ~2.7us) and
                # the DVE copies (~10us); the sync ring is idle mid-kernel and
                # the framework epilogue cost is flat in DMA count.  Only off
                # the endgame path (latency ~1.5us vs ~0.3us for PE).
                nc.sync.dma_start(
                    aot[:, b, KC * qq:KC * (qq + 1)],
                    ao_q[j][qq][:, 128 * b:128 * (b + 1)],
                    transpose=True,
                )

            yv = y.rearrange("(c p) d -> p c d", p=128)

            def out_oproj(j, aot, tt4, on_scalar=False):
                # ost (and y) are f16: halves the output DMA bytes; the host
                # sums the four per-batch partials in f32.
                ost = ostpool.tile([128, D], F16, tag="ost",
                                   name=f"ost{j}_{tt4}")
                for n in range(2):
                    ps_o = ptp.tile([128, QC], F32, tag="t",
                                    name=f"po{j}_{tt4}_{n}")
                    for b in range(2):
                        with nc.named_scope("mm_oproj"):
                            nc.tensor.matmul(
                                ps_o[:],
                                aot[:, b, KC * tt4:KC * (tt4 + 1)],
                                wo_sb[:, b, 512 * n:512 * (n + 1)],
                                start=(b == 0), stop=(b == 1),
                            )
                        yield
                    if on_scalar:
                        # ACT is idle after its last exp; draining the final
                        # output tiles there keeps the endgame's DVE chain
                        # (norms) and the drain in parallel.  The two halves
                        # ship on different rings so their gens overlap.
                        nc.scalar.copy(ost[:, 512 * n:512 * (n + 1)], ps_o[:])
                        ring = nc.sync if n == 0 else nc.scalar
                        ring.dma_start(
                            yv[:, 4 * j + tt4, 512 * n:512 * (n + 1)],
                            ost[:, 512 * n:512 * (n + 1)],
                        )
                    else:
                        nc.vector.tensor_copy(ost[:, 512 * n:512 * (n + 1)], ps_o[:])
                if not on_scalar:
                    nc.sync.dma_start(yv[:, 4 * j + tt4, :], ost[:])

            def out_gen(j):
                aot = aotpool.tile([128, 2, QC], F16, tag="aot", name=f"aot{j}")
                for qq in range(4):
                    for b in range(2):
                        out_tpose_dma(j, aot, qq, b)
                    yield
                for tt4 in range(QC // KC):
                    yield from out_oproj(j, aot, tt4)

            # ---- interleaved schedule ----
            # projqk(0, f=0) runs alone (nothing to overlap yet); each
            # attention unit's QK+exp stream is then interleaved with
            # (a) the previous unit's AV matmuls (pumped fast enough to
            # finish within the unit, avoiding a boundary drain) and
            # (b) "big" filler work from a FIFO of generators: the
            # remaining projections and the previous chunks' output
            # projections, keeping the PE busy while ACT runs exp.
            # Forced drains at unit boundaries guarantee emission-order
            # prerequisites (the dependency graph is defined by emission
            # order).  The last unit gets a custom endgame: its own AV
            # groups and out(3) chase the final exps instead of running
            # serially after them.
            work = []          # FIFO of big-step generators

            def pump_work(n):
                while n > 0 and work:
                    try:
                        next(work[0])
                        n -= 1
                    except StopIteration:
                        work.pop(0)

            def drain(g):
                if g in work:
                    work.remove(g)
                for _ in g:
                    pass

            for _ in projqk_gen(0, 0):
                pass
            pqk = {}
            pv = {}
            pqk[(0, 1)] = projqk_gen(0, 1)
            pv[0] = projv_gen(0)
            work.append(pqk[(0, 1)])
            work.append(pv[0])

            units = [(j, pair) for j in range(NQ) for pair in range(2)]
            prev_av = [None]

            def pump_av(n):
                k = 0
                while k < n and prev_av[0] is not None:
                    try:
                        next(prev_av[0])
                        k += 1
                    except StopIteration:
                        prev_av[0] = None
                return k

            for unit in units:
                j, pair = unit
                last_unit = unit == (NQ - 1, 1)
                if pair == 0 and j + 1 < NQ:
                    pqk[(j + 1, 0)] = projqk_gen(j + 1, 0)
                    pqk[(j + 1, 1)] = projqk_gen(j + 1, 1)
                    pv[j + 1] = projv_gen(j + 1)
                    work.append(pqk[(j + 1, 0)])
                    work.append(pqk[(j + 1, 1)])
                    work.append(pv[j + 1])
                if pair == 1 and j >= 1:
                    # ao_q[j-1] is fully written only once av_gen((j-1, 1))
                    # has drained (end of unit (j, 0)); emitting out_gen
                    # earlier would read-before-write.
                    work.append(out_gen(j - 1))
                ni = 4 * j + 4
                for i in range(ni - 1 if last_unit else ni):
                    # Filler (long 213ns projection streams) goes right
                    # BEFORE the qk pair so its 64-row LDWEIGHTS hides
                    # under a long stream instead of trailing the dense
                    # 27ns AV burst (whose LDW port is saturated).
                    pump_work(2)
                    emit_A(unit, i)
                    adv = pump_av(10)
                    pump_work((10 - adv) // 3)
                while prev_av[0] is not None:
                    pump_av(64)
                if pair == 0:
                    # unit (j, 1) needs qt/kt f=1; av_gen((j, 0)) (pumped
                    # during (j, 1)) reads v chunks of chunk j.
                    drain(pqk[(j, 1)])
                    drain(pv[j])
                else:
                    if (j + 1, 0) in pqk:
                        drain(pqk[(j + 1, 0)])
                prev_av[0] = av_gen(unit)
            # Endgame for unit (3, 1): exps 0..14 ran in the normal loop;
            # only exp 15 (the last diagonal block) is outstanding, emitted
            # once the av stream has pumped past group qq=0 h0 (by which
            # point exp 13's PSUM score buffer is free, so qk15 never
            # stalls the PE queue).  As each query subchunk completes, its
            # second-pair transpose and output-projection slice chase it —
            # after the last exp only the true dependency chain of the
            # final subchunk remains.
            j3 = NQ - 1
            aot3 = aotpool.tile([128, 2, QC], F16, tag="aot", name="aot3")
            for qq in range(4):
                # PE transposes here: a DMA version queues behind
                # out_gen(2)'s ring traffic and arrives too late,
                # serializing the endgame oprojs (measured +8us).
                out_tpose(j3, aot3, qq, 0)
            av31 = prev_av[0]
            steps = 0
            for ev in av31:
                steps += 1
                if steps == 13:
                    emit_A((j3, 1), 4 * j3 + 3)
                if isinstance(ev, tuple) and ev[0] == "qqdone":
                    qq = ev[1]
                    out_tpose(j3, aot3, qq, 1, on_scalar=True)
                    for _ in out_oproj(j3, aot3, qq, on_scalar=True):
                        pass
                pump_work(2)
            for g in list(work):
                drain(g)
    nc.compile()
    return nc


def _get_nc():
    if "nc" not in _CACHE:
        _CACHE["nc"] = build_nc()
    return _CACHE["nc"]


def make_in_maps(x, q_W, k_W, v_W, o_W):
    x = np.asarray(x, dtype=np.float32)
    in_maps = []
    xTs = [np.ascontiguousarray(x[b].T).astype(np.float16) for b in range(2)]
    for c in range(8):
        b, g = c // 4, c % 4
        fs = slice(CF * g, CF * (g + 1))
        in_maps.append({
            "xT": xTs[b],
            "wq": np.ascontiguousarray(np.asarray(q_W, np.float32)[fs].T).astype(np.float16),
            "wk": np.ascontiguousarray(np.asarray(k_W, np.float32)[fs].T).astype(np.float16),
            "wv": np.ascontiguousarray(np.asarray(v_W, np.float32)[fs].T).astype(np.float16),
            "wo": np.ascontiguousarray(np.asarray(o_W, np.float32)[:, fs].T).astype(np.float16),
        })
    return in_maps


def kernel(x, q_W, k_W, v_W, o_W, trace=False):
    nc = _get_nc()
    in_maps = make_in_maps(x, q_W, k_W, v_W, o_W)
    res = run_bass_kernel_spmd(nc, in_maps, core_ids=list(range(8)),
                               trace=trace)
    _CACHE["last_results"] = res
    ys = [np.asarray(res.results[c]["y"], np.float32) for c in range(8)]
    out = np.stack([
        ys[0] + ys[1] + ys[2] + ys[3],
        ys[4] + ys[5] + ys[6] + ys[7],
    ]).astype(np.float32)
    return out



# revision 30
# speedup vs baseline: 1.0423x; 1.0015x over previous
"""Multi-head causal attention (B=2, S=2048, D=1024, H=16, hd=64) on 8 TRN2
NeuronCores.

Sharding: 2-way batch x 4-way head tensor parallel. Core c handles batch
c//4 and heads 4*(c%4) .. 4*(c%4)+3 (a 256-column feature slice of the QKV
projections / 256 rows of o_W). Each core computes a full [2048, 1024]
partial of its batch's output; the host sums the 4 partials per batch.

All matmuls in fp16 (fp32 PSUM accumulate; validated end-to-end max rel
error ~5e-4 vs the fp32 reference):
  1. Qt/Kt feature-major:  Qt[f, tok] = sum_D wq[D, f] * xT[D, tok]
  2. V row-major (stationary operand = xT chunk), with a 1.0 column
     appended per head ([tok, 65] blocks) so the attention-value matmul
     also produces the softmax denominator.
  3. Causal attention per (head, 512-query chunk), keys in 128 chunks:
       St[k, q] = Kt_chunk.T @ Qt      (scores transposed; the two heads
                                        of a pair run as concurrent
                                        row-tiled 64-contraction matmuls)
       U = exp(St / 8)                 (ACT, scale fused; no max
                                        subtraction -- scores are O(1))
       causal mask: gpsimd affine_select zeroes k > q on the diagonal
       128x128 block; sub-diagonal key chunks are skipped entirely.
     AV row-major per 128-query subchunk, U slice as the stationary:
       AO_aug[q, 65] += U_i[:, qslice].T @ V_aug[i]   (col 64 = denom)
     Normalize: rec = 1/AO_aug[:, 64] (DVE [128,1]), then
     tensor_scalar multiply (per-partition broadcast).
  4. AO transposed back to feature-major via PE transpose (fp16), then
     y[tok, :] = sum_f AOt[f-chunk, tok].T @ wo[f-chunk, :]

Schedule (the key to the speedup vs a phase-by-phase layout): the
attention exp stream costs ~75us of ACTIVATE, comparable to the ~110us
of total PE matmul work, so everything is software-pipelined into one
continuous stream: per attention unit (query-chunk j, head pair), the
QK+exp emission is interleaved with (a) the previous unit's AV matmuls
(pumped at ~10 steps per exp so they finish within the unit) and
(b) filler work from a generator FIFO -- the next token chunk's QKV
projections and the previous chunk's output projection.  Forced drains
at unit boundaries establish emission-order prerequisites (the Tile
dependency graph is defined by emission order).  The last unit gets a
custom endgame: only exp 15 is held back; the AV groups, second-pair
transposes, and output projection slices chase it so that after the
last exp only the final query subchunk's true dependency chain
remains.

Startup: the input DMA fabric is bandwidth-serialized (~18us for the
full 6MB), so transfers are strictly need-ordered and the critical
prefix gating the first projection matmul (wq/wk f=0 halves + the
first half of token chunk 0) is trimmed to 8KB/partition.  A burst of
dummy matmuls keeps the PE busy (and the HAM clock gate open) through
the ~7us framework preamble + prefix DMA, so the real projections run
at 2.4 GHz from their first instruction.  Non-endgame AO transposes
ride the DMA XBAR (sync ring, idle mid-kernel) instead of the PE; the
y output is f16 (host sums partials in f32), halving the output DMA
bytes.  A ~7.5us framework epilogue (semaphore teardown) after the
last DMA is flat in everything this kernel controls.
"""

import numpy as np

import concourse.mybir as mybir
import concourse.tile as tile
from concourse import bacc
from concourse.bass_utils import run_bass_kernel_spmd

F32 = mybir.dt.float32
F16 = mybir.dt.float16

S = 2048          # tokens per batch (= per core)
D = 1024          # model dim
HD = 64           # head dim
CORE_HEADS = 4    # heads per core
CF = CORE_HEADS * HD  # feature columns per core (256)
QC = 512          # query chunk (QK/exp granularity)
KC = 128          # key chunk
NQ = S // QC      # 4 query chunks
NK = S // KC      # 16 key chunks
ND = D // 128     # 8 contraction chunks

_CACHE = {}


def build_nc():
    nc = bacc.Bacc()
    xT = nc.dram_tensor("xT", [D, S], F16, kind="ExternalInput")
    wq = nc.dram_tensor("wq", [D, CF], F16, kind="ExternalInput")
    wk = nc.dram_tensor("wk", [D, CF], F16, kind="ExternalInput")
    wv = nc.dram_tensor("wv", [D, CF], F16, kind="ExternalInput")
    wo = nc.dram_tensor("wo", [CF, D], F16, kind="ExternalInput")
    y = nc.dram_tensor("y", [S, D], F16, kind="ExternalOutput")

    with tile.TileContext(nc) as tc:
        with (
            tc.tile_pool(name="big", bufs=1) as big,
            tc.tile_pool(name="w", bufs=1) as wpool,
            tc.tile_pool(name="u", bufs=34) as upool,
            tc.tile_pool(name="aoq", bufs=16) as aoqpool,
            tc.tile_pool(name="aot", bufs=3) as aotpool,
            tc.tile_pool(name="sm", bufs=8) as smpool,
            tc.tile_pool(name="ost", bufs=4) as ostpool,
            tc.tile_pool(name="ps", bufs=2, space="PSUM") as psp,
            tc.tile_pool(name="pav", bufs=2, space="PSUM") as pavp,
            tc.tile_pool(name="pt", bufs=2, space="PSUM") as ptp,
        ):
            # ---- constants ----
            ident = wpool.tile([128, 128], F16, tag="ident")
            nc.gpsimd.memset(ident[:], 0.0)
            nc.gpsimd.affine_select(
                out=ident[:], in_=ident[:],
                compare_op=mybir.AluOpType.not_equal, fill=1.0,
                base=0, channel_multiplier=1, pattern=[[-1, 128]],
            )

            # ---- weight + activation loads, startup-ordered ----
            # First projection matmul needs xs[:, :, 0:512] + wq/wk; those
            # gens go first, spread across all four HWDGE/SWDGE rings so the
            # descriptor generations overlap (the two-ring version serialized
            # ~10us of gens on sync+scalar).  scalar's ring frees up before
            # the first exp; wo (needed last) goes via gpsimd SWDGE.
            wq_sb = wpool.tile([128, ND, CF], F16, tag="wq")
            wk_sb = wpool.tile([128, ND, CF], F16, tag="wk")
            wv_sb = wpool.tile([128, ND, CF], F16, tag="wv")
            wo_sb = wpool.tile([128, 2, D], F16, tag="wo")
            xs = big.tile([128, ND, S], F16, tag="xs")
            xTv = xT.rearrange("(n p) m -> p n m", p=128)
            # The DMA fabric is bandwidth-serialized (~0.39 ns per byte per
            # partition aggregate; the full 6MB input takes ~18us), so ring
            # choice doesn't matter for throughput — only FIFO ORDER does.
            # Transfers are strictly need-ordered, and the critical prefix
            # that gates the first real matmul (wq/wk f=0 halves + the
            # first half of token-chunk 0) is trimmed to 8KB/partition
            # (~3.1us), putting real PE work at ~+12us.  Every extra DMA
            # instruction also costs ~0.15us in the framework epilogue
            # (per-DMA semaphore teardown), so transfers are kept coarse.
            wqv = wq.rearrange("(n p) m -> p n m", p=128)
            wkv = wk.rearrange("(n p) m -> p n m", p=128)
            nc.sync.dma_start(wq_sb[:, :, 0:128], wqv[:, :, 0:128])
            nc.scalar.dma_start(wk_sb[:, :, 0:128], wkv[:, :, 0:128])
            nc.sync.dma_start(xs[:, 0:4, 0:QC], xTv[:, 0:4, 0:QC])
            nc.scalar.dma_start(xs[:, 4:8, 0:QC], xTv[:, 4:8, 0:QC])
            nc.sync.dma_start(wq_sb[:, :, 128:CF], wqv[:, :, 128:CF])
            nc.scalar.dma_start(wk_sb[:, :, 128:CF], wkv[:, :, 128:CF])
            nc.scalar.dma_start(wv_sb[:], wv.rearrange("(n p) m -> p n m", p=128))
            nc.sync.dma_start(xs[:, :, QC:2 * QC], xTv[:, :, QC:2 * QC])
            nc.scalar.dma_start(xs[:, :, 2 * QC:3 * QC], xTv[:, :, 2 * QC:3 * QC])
            nc.sync.dma_start(wo_sb[:], wo.rearrange("(b p) n -> p b n", p=128))
            nc.scalar.dma_start(xs[:, :, 3 * QC:4 * QC], xTv[:, :, 3 * QC:4 * QC])

            # qt/kt: [128, 2, S]: partition = feat % 128 (2 heads), block =
            # feat // 128 (head pair), col = token.
            qt = big.tile([128, 2, S], F16, tag="qt")
            kt = big.tile([128, 2, S], F16, tag="kt")
            v_sb = big.tile([128, NK, CORE_HEADS * (HD + 1)], F16, tag="v")
            nc.vector.memset(
                v_sb[:].rearrange("p n (h c) -> p n h c", c=HD + 1)[:, :, :, HD:],
                1.0,
            )

            # ---- PE warm-up: dummy matmuls keep the PE busy (and the HAM
            # clock gate open) through the framework preamble + input DMA
            # (~13us), so the real projections run at 2.4 GHz from their
            # start.
            ps_warm = ptp.tile([128, 64], F32, tag="t", name="ps_warm")
            for _ in range(110):
                nc.tensor.matmul(ps_warm[0:64, :], ident[:, 0:64],
                                 ident[:, 0:64], start=True, stop=True,
                                 skip_group_check=True)

            # ---- projection emitters (generators: yield after each PE op)
            def projqk_gen(t, f):
                ps_q = ptp.tile([128, QC], F32, tag="t", name=f"pq{t}_{f}")
                ps_k = ptp.tile([128, QC], F32, tag="t", name=f"pk{t}_{f}")
                for k in range(ND):
                    with nc.named_scope("mm_projqk"):
                        nc.tensor.matmul(
                            ps_q[:],
                            wq_sb[:, k, 128 * f:128 * (f + 1)],
                            xs[:, k, QC * t:QC * (t + 1)],
                            start=(k == 0), stop=(k == ND - 1),
                        )
                    yield
                    with nc.named_scope("mm_projqk"):
                        nc.tensor.matmul(
                            ps_k[:],
                            wk_sb[:, k, 128 * f:128 * (f + 1)],
                            xs[:, k, QC * t:QC * (t + 1)],
                            start=(k == 0), stop=(k == ND - 1),
                        )
                    yield
                nc.vector.tensor_copy(qt[:, f, QC * t:QC * (t + 1)], ps_q[:])
                nc.vector.tensor_copy(kt[:, f, QC * t:QC * (t + 1)], ps_k[:])

            def projv_gen(t):
                for tt in range(4 * t, 4 * t + 4):
                    ps = ptp.tile([128, CF], F32, tag="t", name=f"pv{tt}")
                    for k in range(ND):
                        with nc.named_scope("mm_projv"):
                            nc.tensor.matmul(
                                ps[:],
                                xs[:, k, KC * tt:KC * (tt + 1)],
                                wv_sb[:, k, :],
                                start=(k == 0), stop=(k == ND - 1),
                            )
                        yield
                    nc.vector.tensor_copy(
                        v_sb[:, tt, :].rearrange("p (h c) -> p h c", c=HD + 1)[:, :, :HD],
                        ps[:].rearrange("p (h c) -> p h c", c=HD),
                    )

            # ---- attention units ----
            units = [(j, pair) for j in range(NQ) for pair in range(2)]
            us = {}
            ao_q = {}

            def emit_A(unit, i):
                # Both heads' scores land in one 2-bank PSUM tile so a
                # single ACTIVATE (and a single affine_select) covers the
                # pair — halves the fixed 352-cycle ACT pipeline overhead.
                # The two 64-contraction matmuls run concurrently via PE
                # row tiling (auto tile_position from base partitions 0/64).
                j, pair = unit
                t = i - 4 * j
                qo = max(0, KC * t)
                w = QC - qo
                ps_s = psp.tile([128, 2, QC], F32, tag="s", name=f"s{j}_{pair}_{i}")
                for hx, h in enumerate((2 * pair, 2 * pair + 1)):
                    hp = 64 * (h % 2)
                    with nc.named_scope("mm_qk"):
                        nc.tensor.matmul(
                            ps_s[:, hx, 0:w],
                            kt[hp:hp + 64, pair, KC * i:KC * (i + 1)],
                            qt[hp:hp + 64, pair, QC * j + qo:QC * (j + 1)],
                            start=True, stop=True,
                            skip_group_check=True,
                        )
                u = upool.tile([128, 2, w], F16, tag="u", name=f"u{j}_{pair}_{i}")
                nc.scalar.activation(
                    u[:], ps_s[:, :, 0:w],
                    mybir.ActivationFunctionType.Exp, scale=0.125,
                )
                if t >= 0:
                    nc.gpsimd.affine_select(
                        out=u[:, :, 0:KC], in_=u[:, :, 0:KC],
                        compare_op=mybir.AluOpType.is_ge, fill=0.0,
                        base=0, channel_multiplier=-1,
                        pattern=[[0, 2], [1, KC]],
                    )
                us[unit, i] = (u, qo)

            def av_gen(unit):
                """Generator: AV matmuls for one unit, yielding after each
                key-chunk step; norms emitted at each query-subchunk's end.
                One PSUM bank per accumulation group (bank-granular
                start/stop semantics)."""
                j, pair = unit
                nk = 4 * j + 4
                if j not in ao_q:
                    ao_q[j] = [aoqpool.tile([128, CF], F16, tag="aoq",
                                            name=f"ao_q{j}_{qq}")
                               for qq in range(4)]
                for qq in range(4):
                    for hx, h in enumerate((2 * pair, 2 * pair + 1)):
                        av = pavp.tile([128, HD + 1], F32, tag="av",
                                       name=f"av{j}_{h}_{qq}")
                        last = 4 * j + qq
                        for i in range(last + 1):
                            u, qo = us[unit, i]
                            with nc.named_scope("mm_av"):
                                nc.tensor.matmul(
                                    av[:],
                                    u[:, hx, KC * qq - qo:KC * (qq + 1) - qo],
                                    v_sb[:, i, 65 * h:65 * h + 65],
                                    start=(i == 0), stop=(i == last),
                                )
                            yield
                        rec = smpool.tile([128, 1], F32, tag="rec",
                                          name=f"rec{j}_{h}_{qq}")
                        nc.vector.reciprocal(rec[:], av[:, HD:HD + 1])
                        nc.vector.tensor_scalar_mul(
                            ao_q[j][qq][:, HD * h:HD * (h + 1)],
                            av[:, 0:HD], rec[:],
                        )
                        yield ("qqdone", qq) if hx == 1 else None
                for i in range(nk):
                    us.pop((unit, i), None)

            def out_tpose(j, aot, qq, b, on_scalar=False):
                ps_t = ptp.tile([128, 128], F16, tag="t",
                                name=f"pt{j}_{qq}_{b}")
                with nc.named_scope("mm_tpose"):
                    nc.tensor.transpose(
                        ps_t[:], ao_q[j][qq][:, 128 * b:128 * (b + 1)],
                        ident[:],
                    )
                if on_scalar:
                    nc.scalar.copy(aot[:, b, KC * qq:KC * (qq + 1)], ps_t[:])
                else:
                    nc.vector.tensor_copy(
                        aot[:, b, KC * qq:KC * (qq + 1)], ps_t[:],
                    )

            def out_tpose_dma(j, aot, qq, b):
                # DMA XBAR transpose: frees the PE (24 transposes ~2.7us) and
                # the DVE copies (~10us); the sync ring is idle mid-kernel and
                # the framework epilogue cost is flat in DMA count.  Only off
                # the endgame path (latency ~1.5us vs ~0.3us for PE).
                nc.sync.dma_start(
                    aot[:, b, KC * qq:KC * (qq + 1)],
                    ao_q[j][qq][:, 128 * b:128 * (b + 1)],
                    transpose=True,
                )

            yv = y.rearrange("(c p) d -> p c d", p=128)

            def out_oproj(j, aot, tt4, on_scalar=False):
                # ost (and y) are f16: halves the output DMA bytes; the host
                # sums the four per-batch partials in f32.
                ost = ostpool.tile([128, D], F16, tag="ost",
                                   name=f"ost{j}_{tt4}")
                for n in range(2):
                    ps_o = ptp.tile([128, QC], F32, tag="t",
                                    name=f"po{j}_{tt4}_{n}")
                    for b in range(2):
                        with nc.named_scope("mm_oproj"):
                            nc.tensor.matmul(
                                ps_o[:],
                                aot[:, b, KC * tt4:KC * (tt4 + 1)],
                                wo_sb[:, b, 512 * n:512 * (n + 1)],
                                start=(b == 0), stop=(b == 1),
                            )
                        yield
                    if on_scalar:
                        # ACT is idle after its last exp; draining the final
                        # output tiles there keeps the endgame's DVE chain
                        # (norms) and the drain in parallel.  The two halves
                        # ship on different rings so their gens overlap.
                        nc.scalar.copy(ost[:, 512 * n:512 * (n + 1)], ps_o[:])
                        ring = nc.sync if n == 0 else nc.scalar
                        ring.dma_start(
                            yv[:, 4 * j + tt4, 512 * n:512 * (n + 1)],
                            ost[:, 512 * n:512 * (n + 1)],
                        )
                    else:
                        nc.vector.tensor_copy(ost[:, 512 * n:512 * (n + 1)], ps_o[:])
                if not on_scalar:
                    nc.sync.dma_start(yv[:, 4 * j + tt4, :], ost[:])

            def out_gen(j):
                aot = aotpool.tile([128, 2, QC], F16, tag="aot", name=f"aot{j}")
                for qq in range(4):
                    for b in range(2):
                        out_tpose_dma(j, aot, qq, b)
                    yield
                for tt4 in range(QC // KC):
                    yield from out_oproj(j, aot, tt4)

            # ---- interleaved schedule ----
            # projqk(0, f=0) runs alone (nothing to overlap yet); each
            # attention unit's QK+exp stream is then interleaved with
            # (a) the previous unit's AV matmuls (pumped fast enough to
            # finish within the unit, avoiding a boundary drain) and
            # (b) "big" filler work from a FIFO of generators: the
            # remaining projections and the previous chunks' output
            # projections, keeping the PE busy while ACT runs exp.
            # Forced drains at unit boundaries guarantee emission-order
            # prerequisites (the dependency graph is defined by emission
            # order).  The last unit gets a custom endgame: its own AV
            # groups and out(3) chase the final exps instead of running
            # serially after them.
            work = []          # FIFO of big-step generators

            def pump_work(n):
                while n > 0 and work:
                    try:
                        next(work[0])
                        n -= 1
                    except StopIteration:
                        work.pop(0)

            def drain(g):
                if g in work:
                    work.remove(g)
                for _ in g:
                    pass

            for _ in projqk_gen(0, 0):
                pass
            pqk = {}
            pv = {}
            pqk[(0, 1)] = projqk_gen(0, 1)
            pv[0] = projv_gen(0)
            work.append(pqk[(0, 1)])
            work.append(pv[0])

            units = [(j, pair) for j in range(NQ) for pair in range(2)]
            prev_av = [None]

            def pump_av(n):
                k = 0
                while k < n and prev_av[0] is not None:
                    try:
                        next(prev_av[0])
                        k += 1
                    except StopIteration:
                        prev_av[0] = None
                return k

            for unit in units:
                j, pair = unit
                last_unit = unit == (NQ - 1, 1)
                if pair == 0 and j + 1 < NQ:
                    pqk[(j + 1, 0)] = projqk_gen(j + 1, 0)
                    pqk[(j + 1, 1)] = projqk_gen(j + 1, 1)
                    pv[j + 1] = projv_gen(j + 1)
                    work.append(pqk[(j + 1, 0)])
                    work.append(pqk[(j + 1, 1)])
                    work.append(pv[j + 1])
                if pair == 1 and j >= 1:
                    # ao_q[j-1] is fully written only once av_gen((j-1, 1))
                    # has drained (end of unit (j, 0)); emitting out_gen
                    # earlier would read-before-write.
                    work.append(out_gen(j - 1))
                ni = 4 * j + 4
                for i in range(ni - 1 if last_unit else ni):
                    # Filler (long 213ns projection streams) goes right
                    # BEFORE the qk pair so its 64-row LDWEIGHTS hides
                    # under a long stream instead of trailing the dense
                    # 27ns AV burst (whose LDW port is saturated).
                    # Exception: unit (0,0) keeps filler AFTER the pair —
                    # its fillers read wq/wk f=1, which the serialized DMA
                    # stream only delivers mid-unit; pulling them early
                    # stalls the in-order PE queue (~0.6us measured).
                    if unit != (0, 0):
                        pump_work(2)
                    emit_A(unit, i)
                    adv = pump_av(10)
                    if unit == (0, 0):
                        pump_work(2)
                    pump_work((10 - adv) // 3)
                while prev_av[0] is not None:
                    pump_av(64)
                if pair == 0:
                    # unit (j, 1) needs qt/kt f=1; av_gen((j, 0)) (pumped
                    # during (j, 1)) reads v chunks of chunk j.
                    drain(pqk[(j, 1)])
                    drain(pv[j])
                else:
                    if (j + 1, 0) in pqk:
                        drain(pqk[(j + 1, 0)])
                prev_av[0] = av_gen(unit)
            # Endgame for unit (3, 1): exps 0..14 ran in the normal loop;
            # only exp 15 (the last diagonal block) is outstanding, emitted
            # once the av stream has pumped past group qq=0 h0 (by which
            # point exp 13's PSUM score buffer is free, so qk15 never
            # stalls the PE queue).  As each query subchunk completes, its
            # second-pair transpose and output-projection slice chase it —
            # after the last exp only the true dependency chain of the
            # final subchunk remains.
            j3 = NQ - 1
            aot3 = aotpool.tile([128, 2, QC], F16, tag="aot", name="aot3")
            for qq in range(4):
                # PE transposes here: a DMA version queues behind
                # out_gen(2)'s ring traffic and arrives too late,
                # serializing the endgame oprojs (measured +8us).
                out_tpose(j3, aot3, qq, 0)
            av31 = prev_av[0]
            steps = 0
            for ev in av31:
                steps += 1
                if steps == 13:
                    emit_A((j3, 1), 4 * j3 + 3)
                if isinstance(ev, tuple) and ev[0] == "qqdone":
                    qq = ev[1]
                    out_tpose(j3, aot3, qq, 1, on_scalar=True)
                    for _ in out_oproj(j3, aot3, qq, on_scalar=True):
                        pass
                pump_work(2)
            for g in list(work):
                drain(g)
    nc.compile()
    return nc


def _get_nc():
    if "nc" not in _CACHE:
        _CACHE["nc"] = build_nc()
    return _CACHE["nc"]


def make_in_maps(x, q_W, k_W, v_W, o_W):
    x = np.asarray(x, dtype=np.float32)
    in_maps = []
    xTs = [np.ascontiguousarray(x[b].T).astype(np.float16) for b in range(2)]
    for c in range(8):
        b, g = c // 4, c % 4
        fs = slice(CF * g, CF * (g + 1))
        in_maps.append({
            "xT": xTs[b],
            "wq": np.ascontiguousarray(np.asarray(q_W, np.float32)[fs].T).astype(np.float16),
            "wk": np.ascontiguousarray(np.asarray(k_W, np.float32)[fs].T).astype(np.float16),
            "wv": np.ascontiguousarray(np.asarray(v_W, np.float32)[fs].T).astype(np.float16),
            "wo": np.ascontiguousarray(np.asarray(o_W, np.float32)[:, fs].T).astype(np.float16),
        })
    return in_maps


def kernel(x, q_W, k_W, v_W, o_W, trace=False):
    nc = _get_nc()
    in_maps = make_in_maps(x, q_W, k_W, v_W, o_W)
    res = run_bass_kernel_spmd(nc, in_maps, core_ids=list(range(8)),
                               trace=trace)
    _CACHE["last_results"] = res
    ys = [np.asarray(res.results[c]["y"], np.float32) for c in range(8)]
    out = np.stack([
        ys[0] + ys[1] + ys[2] + ys[3],
        ys[4] + ys[5] + ys[6] + ys[7],
    ]).astype(np.float32)
    return out

